# revision 1
# baseline (speedup 1.0000x reference)
"""CRF forward-algorithm loss on 8 Trainium2 NeuronCores.

Math: the reference does, per step t (8192 steps, K=2048 tags):
    fv'[n] = logsumexp_p(fv[p] + T[n,p]) + h[t,n]
and finally logsumexp(fv + T[END]).

We run the recurrence in LINEAR space: with w ~ exp(fv) (rescaled each step),
    m[n]  = sum_p expT[n,p] * w[p]          (matvec, tensor engine, bf16)
    S     = sum_n m[n]                      (computed as an extra matvec column
                                             carrying colsum(expT))
    w'[n] = (m[n] / S) * exp(h[t,n])
    record 1/S
answer = log(sum_p expT[END,p] * w_final[p]) - sum_t log(recip_t)

Distribution: tensor-parallel over the `next` axis. Each core holds the
[2048 prev x 256 next] slice of expT^T as the matmul MOVING operand (resident
in SBUF, bf16) and computes a 256-slice of m per step; slices are exchanged
every step with direct core-to-core SBUF remote DMA broadcasts (XOR slot
scheme keeps the program SPMD-uniform: receiver r's gather slot k holds data
from core r^k; all per-core tensors are laid out by the host accordingly).

Per-core, per-step pipeline (raw bass, hand-scheduled, monotonic semaphores):
  PE   : 16 accumulating matmuls [128c x 1] x [128c x 257] -> psum row [1,257]
         + 2 tiny matmuls that transpose the scaled row into [128,2]
  DVE  : 1/S, scale row, copy transposed cols to send tile + own gather slot,
         w' = gather * expH  (bf16)
  ACT  : expH = exp(h-tile)  ([128,16], streamed from HBM in blocks)
  GPSIMD: 7 remote_dma_broadcast preps + trigger (data), credit broadcast
  SP   : h block prefetch DMA
"""

import sys

if "/opt/trn_rl_repo" not in sys.path:
    sys.path.insert(0, "/opt/trn_rl_repo")

import numpy as np
import ml_dtypes

import concourse.bass as bass
import concourse.bacc as bacc
import concourse.mybir as mybir

START_IDX = 0
END_IDX = 1
K = 2048
SEQ = 8192
NCORES = 8
P = 128
SLICE = K // NCORES          # 256 nexts per core
MT = K // P                  # 16 contract chunks of 128
MCOLS = SLICE + 1            # 256 nexts + 1 colsum column
BF16 = mybir.dt.bfloat16
F32 = mybir.dt.float32
NPBF16 = ml_dtypes.bfloat16


def build_bass(
    seq_blocks: int,
    blk_steps: int,
    dbg_delay: int = 0,
    variant: str = "full",  # timing diagnostics: full | nocomm | notrans | mmonly
) -> bass.Bass:
    """Device program. seq = seq_blocks * blk_steps, blk_steps must be even."""
    assert blk_steps % 2 == 0
    comm = variant == "full"
    trans = variant in ("full", "nocomm")
    dvework = variant != "mmonly"
    seq = seq_blocks * blk_steps
    nc = bacc.Bacc(None, target_bir_lowering=False, num_devices=NCORES)

    movq = nc.declare_dram_parameter("movq", [P, MT * MCOLS], BF16, isOutput=False)
    hq = nc.declare_dram_parameter("hq", [P, seq * MT], F32, isOutput=False)
    winit = nc.declare_dram_parameter("winit", [P, MT], BF16, isOutput=False)
    wout = nc.declare_dram_parameter("wout", [P, MT], BF16, isOutput=True)
    rec_out = nc.declare_dram_parameter("rec", [1, seq], BF16, isOutput=True)

    movsb = nc.alloc_sbuf_tensor("movsb", [P, MT * MCOLS], BF16)
    w_sb = nc.alloc_sbuf_tensor("w_sb", [P, MT], BF16)
    hq_sb = nc.alloc_sbuf_tensor("hq_sb", [P, 2 * blk_steps * MT], F32)  # holds exp(h)
    graw = nc.alloc_sbuf_tensor("graw", [P, 2 * MT], F32)      # parity halves
    sendt = nc.alloc_sbuf_tensor("sendt", [P, 4], F32)         # parity 2+2
    rawrow = nc.alloc_sbuf_tensor("rawrow", [1, SLICE], BF16)  # unscaled matvec row
    one_sb = nc.alloc_sbuf_tensor("one_sb", [1, 1], F32)
    rec32_tmp = nc.alloc_sbuf_tensor("rec32_tmp", [1, 1], F32)
    rec_sb = nc.alloc_sbuf_tensor("rec_sb", [1, seq], BF16)    # 1/S record (= scale applied)

    psum_m = nc.alloc_psum_tensor("psum_m", [P, 512], F32)     # row 0 used
    psum_ta = nc.alloc_psum_tensor("psum_ta", [P, 512], F32)   # col 0 used
    psum_tb = nc.alloc_psum_tensor("psum_tb", [P, 512], F32)

    # semaphores
    sem_mm = nc.alloc_semaphore("sem_mm")        # PE matvec done     +1/step
    sem_row = nc.alloc_semaphore("sem_row")      # 1/S ready (DVE)    +1/step
    sem_raw = nc.alloc_semaphore("sem_raw")      # raw row copied (ACT) +1/step
    sem_tp = nc.alloc_semaphore("sem_tp")        # transposes done    +1/step
    sem_send = nc.alloc_semaphore("sem_send")    # send tile ready    +1/step
    sem_wdone = nc.alloc_semaphore("sem_wdone")  # w' ready           +1/step
    rsem = [nc.alloc_semaphore(f"rsem{i}") for i in range(2)]   # +16/same-parity step (remote)
    lsem = nc.alloc_semaphore("lsem")            # data send local    +16/step
    psem_d = nc.alloc_semaphore("psem_d")        # data descs written +1/step
    dma0 = nc.alloc_semaphore("dma0")            # prologue loads
    hqsem = [nc.alloc_semaphore(f"hqsem{i}") for i in range(2)]  # h DMAs, +16/same-parity block

    pe, dve, act, gp, sp = nc.tensor, nc.vector, nc.scalar, nc.gpsimd, nc.sync

    # ---- prologue ----
    gp.memset(one_sb[:, :], 1.0)
    sp.dma_start(out=movsb[:, :], in_=movq[:, :]).then_inc(dma0, 16)
    sp.dma_start(out=w_sb[:, :], in_=winit[:, :]).then_inc(dma0, 16)
    sp.dma_start(out=hq_sb[:, 0 : blk_steps * MT], in_=hq[:, 0 : blk_steps * MT]).then_inc(hqsem[0], 16)
    if seq_blocks > 1:
        sp.dma_start(
            out=hq_sb[:, blk_steps * MT : 2 * blk_steps * MT],
            in_=hq[:, blk_steps * MT : 2 * blk_steps * MT],
        ).then_inc(hqsem[1], 16)
    pe.wait_ge(dma0, 32)
    # no remote traffic may be emitted before every core has loaded + zeroed state
    nc.all_core_barrier()

    # ---- per-engine monotonic threshold registers ----
    def reg(engine, name, val=0):
        r = engine.alloc_register(name)
        engine.reg_mov(r, val)
        return r

    pe_wd = reg(pe, "pe_wd")
    pe_row = reg(pe, "pe_row")
    pe_raw = reg(pe, "pe_raw")
    pe_rec = reg(pe, "pe_rec")     # rhs offset into rec_sb
    v_mm = reg(dve, "v_mm")
    v_tp = reg(dve, "v_tp")
    a_mm = reg(act, "a_mm")
    v_rs = [reg(dve, f"v_rs{i}") for i in range(2)]
    v_ls = reg(dve, "v_ls")
    v_rec = reg(dve, "v_rec")      # record write offset (elements)
    v_hq = reg(dve, "v_hq")        # exp(h) tile read offset
    v_hqs = reg(dve, "v_hqs")      # hq block threshold
    g_send = reg(gp, "g_send")
    g_pd = reg(gp, "g_pd")
    s_src = reg(sp, "s_src")
    s_tmp = reg(sp, "s_tmp")
    s_cond = reg(sp, "s_cond")
    # per-parity broadcast dest offsets: graw col 2*my_core_id within the half
    g_off = [gp.alloc_register(f"g_off{i}") for i in range(2)]
    gp.reg_alu(g_off[0], gp.partition_id(), 2, op=mybir.AluOpType.mult)
    gp.reg_add(g_off[1], g_off[0], MT)

    def emit_bcast_prep(par: int):
        gp.remote_dma_broadcast(
            out_ap=bass.AP(graw, g_off[par], [[2 * MT, P], [1, 2]]),
            in_ap=sendt[:, 2 * par : 2 * par + 2],
            remote_sem=rsem[par],
            local_sem=lsem,
            rdests=[(0, k) for k in range(NCORES)],
        ).then_inc(psem_d, 1)

    def emit_step(par: int):
        # ---------------- PE ----------------
        pe.wait_ge(sem_wdone, pe_wd)
        pe.reg_add(pe_wd, pe_wd, 1)
        for j2 in range(MT):
            pe.matmul(
                psum_m[0:1, 0:MCOLS],
                w_sb[:, j2 : j2 + 1],
                movsb[:, j2 * MCOLS : (j2 + 1) * MCOLS],
                start=(j2 == 0),
                stop=(j2 == MT - 1),
            ).then_maybe_inc((sem_mm, 1) if j2 == MT - 1 else None)
        if trans:
            # transposes apply the 1/S scale for free: out = rawrow.T @ recip
            pe.reg_add(pe_row, pe_row, 1)
            pe.wait_ge(sem_row, pe_row)
            pe.reg_add(pe_raw, pe_raw, 1)
            pe.wait_ge(sem_raw, pe_raw)
            rec_pe_ap = bass.AP(rec_sb, pe_rec, [[seq, 1], [1, 1]])
            pe.matmul(psum_ta[0:P, 0:1], rawrow[0:1, 0:P], rec_pe_ap, start=True, stop=True)
            pe.matmul(psum_tb[0:P, 0:1], rawrow[0:1, P : 2 * P], rec_pe_ap, start=True, stop=True).then_inc(sem_tp, 1)
            pe.reg_add(pe_rec, pe_rec, 1)

        # ---------------- ACT: raw row copy (parallel with DVE divide) ------
        if trans:
            act.reg_add(a_mm, a_mm, 1)
            act.wait_ge(sem_mm, a_mm)
            act.activation(
                rawrow[0:1, 0:SLICE], psum_m[0:1, 0:SLICE],
                mybir.ActivationFunctionType.Copy,
            ).then_inc(sem_raw, 1)

        # ---------------- DVE ----------------
        dve.reg_add(v_mm, v_mm, 1)
        dve.wait_ge(sem_mm, v_mm)
        if dvework:
            rec_ap = bass.AP(rec_sb, v_rec, [[seq, 1], [1, 1]])
            dve.reciprocal(rec32_tmp[0:1, 0:1], psum_m[0:1, SLICE : SLICE + 1])
            dve.drain()
            dve.tensor_copy(rec_ap, rec32_tmp[0:1, 0:1]).then_inc(sem_row, 1)
            dve.reg_add(v_rec, v_rec, 1)
        if trans:
            dve.reg_add(v_tp, v_tp, 1)
            dve.wait_ge(sem_tp, v_tp)
            if comm:
                dve.wait_ge(lsem, v_ls)  # my sends through t-1 left sendt
                dve.reg_add(v_ls, v_ls, 16)
            dve.tensor_copy(sendt[:, 2 * par : 2 * par + 1], psum_ta[0:P, 0:1])
            dve.tensor_copy(sendt[:, 2 * par + 1 : 2 * par + 2], psum_tb[0:P, 0:1]).then_inc(sem_send, 1)
            if not comm:
                # own slice locally (normally delivered by the self-dest broadcast)
                dve.tensor_copy(graw[:, MT * par : MT * par + 1], psum_ta[0:P, 0:1])
                dve.tensor_copy(graw[:, MT * par + 1 : MT * par + 2], psum_tb[0:P, 0:1])
        if comm:
            dve.reg_add(v_rs[par], v_rs[par], 16)
            dve.wait_ge(rsem[par], v_rs[par])
        if dbg_delay:
            dve.nop(cycle_cnt=dbg_delay)
        if not comm:
            dve.drain()  # graw written by DVE copies in nocomm variants
        dve.tensor_tensor(
            w_sb[:, :],
            graw[:, MT * par : MT * (par + 1)],
            bass.AP(hq_sb, v_hq, [[2 * blk_steps * MT, P], [1, MT]]),
            op=mybir.AluOpType.mult,
        ).then_inc(sem_wdone, 1)
        dve.reg_add(v_hq, v_hq, MT)

        # ---------------- GPSIMD ----------------
        # Prep this step's frame at the top (Q7 desc-gen overlaps the matvec;
        # source reads are deferred to the trigger), then trigger once the
        # send tile is ready. Buffer flow control is implied by the rsem
        # dependency chain: my trigger(t) happens-after every peer's w-update
        # of step t-2 (their send(t-1) required it) — no explicit credits.
        if comm:
            emit_bcast_prep(par)
            gp.reg_add(g_send, g_send, 1)
            gp.wait_ge(sem_send, g_send)
            gp.reg_add(g_pd, g_pd, 1)
            gp.wait_ge(psem_d, g_pd)
            gp.trigger_dma(count=1)

    # ---- main loop: superblocks of two h-blocks (static buffer parity) ----
    assert seq_blocks % 2 == 0
    with nc.Fori(0, seq_blocks // 2) as g:
        for p01 in range(2):  # h-block index blk = 2*g + p01, buffer half p01
            # DVE: gate on this block's h DMA; read offset = p01 half
            dve.reg_alu(v_hqs, g, 16, op=mybir.AluOpType.mult)
            dve.reg_add(v_hqs, v_hqs, 16)
            dve.wait_ge(hqsem[p01], v_hqs)
            dve.reg_mov(v_hq, p01 * blk_steps * MT)

            # SP: once DVE finishes block 2g+p01, prefetch block 2g+2+p01
            if seq_blocks > 2:
                sp.reg_alu(s_tmp, g, 2 * blk_steps, op=mybir.AluOpType.mult)
                sp.reg_add(s_tmp, s_tmp, (1 + p01) * blk_steps)
                sp.reg_mov(s_cond, 0)
                sp.reg_add(s_cond, g, 0)
                with sp.If_lt(s_cond, seq_blocks // 2 - 1):
                    sp.wait_ge(sem_wdone, s_tmp)
                    sp.reg_add(s_src, g, 0)
                    sp.reg_alu(s_src, s_src, 2 * blk_steps * MT, op=mybir.AluOpType.mult)
                    sp.reg_add(s_src, s_src, (2 + p01) * blk_steps * MT)
                    sp.dma_start(
                        out=bass.AP(
                            hq_sb,
                            p01 * blk_steps * MT,
                            [[2 * blk_steps * MT, P], [1, blk_steps * MT]],
                        ),
                        in_=bass.AP(hq, s_src, [[seq * MT, P], [1, blk_steps * MT]]),
                    ).then_inc(hqsem[p01], 16)

            with nc.Fori(0, blk_steps // 2):
                emit_step(0)
                emit_step(1)

    # ---- epilogue ----
    if dvework:
        sp.wait_ge(sem_row, seq)
        sp.dma_start(out=rec_out[:, :], in_=rec_sb[:, :]).then_inc(dma0, 16)
    sp.wait_ge(sem_wdone, seq)
    sp.dma_start(out=wout[:, :], in_=w_sb[:, :]).then_inc(dma0, 16)
    sp.wait_ge(dma0, 64 if dvework else 48)
    if comm:
        gp.wait_ge(lsem, 16 * seq)
    nc.all_core_barrier()
    nc.finalize()
    return nc


def prep_inputs(h: np.ndarray, transitions: np.ndarray, seq: int):
    """Host-side layout of per-core inputs.

    Gather slot s on every receiver holds sender s's slice (the register-AP
    broadcast writes to graw[:, 2*sender_id : +2]), so the matvec position
    j = col*128 + q maps to global prev index j for every core (identity).
    """
    h32 = np.ascontiguousarray(h.astype(np.float32)[:seq])
    expT32 = np.exp(transitions.astype(np.float32))
    expTq = expT32.astype(NPBF16)
    colsum = expTq.astype(np.float32).sum(axis=0).astype(NPBF16)

    # exp(h) in [q, t, c] layout: hq0[q, t, c] = exp(h[t, c*128 + q]); shared
    hq0 = np.ascontiguousarray(
        np.exp(h32).reshape(seq, MT, P).transpose(2, 0, 1).reshape(P, seq * MT)
    )
    # winit: global prev START=0 sits at (q=0, col=0); identical on all cores
    wi = np.zeros((P, MT), dtype=NPBF16)
    wi[0, 0] = 1.0

    in_maps = []
    for r in range(NCORES):
        # moving tiles: mov[q, j2*257 + col]; col<256 -> expTq[256r+col, j2*128+q]
        A = expTq[256 * r : 256 * (r + 1), :]                    # [256 next, 2048 prev]
        B = np.ascontiguousarray(A.reshape(SLICE, MT, P).transpose(2, 1, 0))  # [q, j2, col]
        C = colsum.reshape(MT, P).T                              # [q, j2]
        mov = np.concatenate([B, C[:, :, None]], axis=2)         # [128, 16, 257]
        mov = np.ascontiguousarray(mov.reshape(P, MT * MCOLS))
        in_maps.append({"movq": mov, "hq": hq0, "winit": wi})
    return in_maps, expT32


def finalize(results, transitions, seq: int):
    """Combine device outputs into the scalar answer (host, fp64)."""
    rec = results[0]["rec"].reshape(-1).astype(np.float64)       # recip values (bf16-exact)
    wfin = results[0]["wout"].astype(np.float64)                 # [128, 16] on core 0
    w_full = wfin.T.reshape(-1)                                  # w_full[c*128+q]
    exp_end = np.exp(transitions[END_IDX].astype(np.float64))
    mterm = float(np.dot(exp_end, w_full))
    ans = np.log(mterm) - np.sum(np.log(rec))
    return np.float32(ans)


def kernel(h: np.ndarray, transitions: np.ndarray) -> np.ndarray:
    from concourse.bass_utils import run_bass_kernel_spmd

    seq_blocks, blk_steps = 16, SEQ // 16
    nc = build_bass(seq_blocks, blk_steps)
    in_maps, _ = prep_inputs(np.asarray(h), np.asarray(transitions), SEQ)
    res = run_bass_kernel_spmd(nc, in_maps, core_ids=list(range(NCORES)))
    return finalize(res.results, np.asarray(transitions), SEQ)


if __name__ == "__main__":
    import reference

    inputs = {k: np.asarray(v) for k, v in reference.setup_inputs().items()}
    out = kernel(**inputs)
    print("kernel:", out)

